# revision 17
# baseline (speedup 1.0000x reference)
"""Multi-head attention (B=4, S=2048, D=512, H=8) on 8 Trainium2 cores.

Sharding: core c = (batch b = c//2, query-half = c%2). Each core computes
1024 query rows of one batch over all 2048 keys and all 8 heads, producing
a disjoint slice of the output -> no inter-core reduction needed.

Per-core layout is fully "transposed land" (contraction dim on partitions):
  xT [512,1024], yT [512,2048] prepared (transposed, bf16, k-tile-packed)
  on host.
  QT = Wq^T @ xT   (Wq pre-scaled by depth^-0.5 on host)
  KT = Wk^T @ yT
  V  = y @ Wv in natural [keys, dim] layout, stored strided into
       V_aug [128, 8*65] with a ones column per head (row 64 of the
       attention matmul output then accumulates softmax denominators).
  per head pair (2p, 2p+1): head A lives on partitions 0:64, head B on
       64:128 of the shared KT/QT tile, so their logits matmuls target
       disjoint PE row groups and run concurrently.
       logitsT[kt] = (KT tile)^T @ QT  (bf16 operands, fp32 PSUM)
       -> one exp over [128, 1024] (ScalarE, PSUM -> SBUF bf16)
       -> attnT += V_aug^T @ PT, fp32 PSUM, accumulated over 16 key tiles.
  normalize: evacuate PSUM fast (DVE copies release the accumulation
       banks so the next pair keeps the PE busy); the denominator row is
       copied to partition 0 of a small tile where the partition-0-only
       custom DVE reciprocal_approx_fast (~51 ULP, ~5x faster than the
       iterative reciprocal) reads it; gpsimd partition_broadcast and the
       multiply run off the critical path.
  out = attnT^T @ Wo -> DMA (fp32).

DMA: every input arrives pre-packed as [128, k-tiles * cols] so each
tensor is ONE transfer with 2-8KB contiguous rows sprayed across all 16
queues (1KB rows cost ~4x more in descriptor overhead), emitted in
consumption order (wv, yT, wk, wq, xT, wo). A ~3.4us burst of dummy
matmuls on the first-arriving tensor warms the PE HAM clock to 2.4 GHz
before the real projections start.

Softmax skips max-subtraction (logits ~ N(0,1); exp cannot overflow fp32).
End-to-end RMS relative error vs fp32 ~5e-3.
"""

import numpy as np
import ml_dtypes

import concourse.bass as bass
import concourse.tile as tile
from concourse import bacc, mybir
from concourse.bass_utils import run_bass_kernel_spmd

F32 = mybir.dt.float32
BF16 = mybir.dt.bfloat16
EXP = mybir.ActivationFunctionType.Exp

B, S, D = 4, 2048, 512
H = 8
DEPTH = D // H  # 64
SQ = S // 2  # queries per core (1024)
SK = S  # keys per core (2048)
N_CORES = 8

P = 128
KT4 = D // P  # 4 contraction tiles for projections
NKT = SK // P  # 16 key tiles
NQT = SQ // P  # 8 query tiles
VAUG_W = H * (DEPTH + 1)  # 520
HKB = SK // 2  # 1024 keys per yT half


def _mm(nc, out, lhsT, rhs, start, stop):
    nc.tensor.matmul(out, lhsT, rhs, start=start, stop=stop)


def build_nc():
    nc = bacc.Bacc("TRN2", target_bir_lowering=False, debug=False)

    xT = nc.dram_tensor("xT", [P, KT4 * SQ], BF16, kind="ExternalInput").ap()
    ytA = nc.dram_tensor("ytA", [P, KT4 * HKB], BF16, kind="ExternalInput").ap()
    ytB = nc.dram_tensor("ytB", [P, KT4 * HKB], BF16, kind="ExternalInput").ap()
    wq = nc.dram_tensor("wq", [P, KT4 * D], BF16, kind="ExternalInput").ap()
    wk = nc.dram_tensor("wk", [P, KT4 * D], BF16, kind="ExternalInput").ap()
    wv = nc.dram_tensor("wv", [P, KT4 * D], BF16, kind="ExternalInput").ap()
    wo = nc.dram_tensor("wo", [P, KT4 * D], BF16, kind="ExternalInput").ap()
    # Output is shipped bf16 (half the DMA bytes of the fp32 result; the
    # host casts back to fp32 -- adds ~0.2% RMS, well inside the budget).
    out = nc.dram_tensor("out", [SQ, D], BF16, kind="ExternalOutput").ap()

    with tile.TileContext(nc) as tc:
        with (
            tc.tile_pool(name="acts", bufs=1) as apool,
            tc.tile_pool(name="ps", bufs=1, space="PSUM") as pspool,
            tc.tile_pool(name="pt", bufs=6) as ptpool,
            tc.tile_pool(name="small", bufs=2) as spool,
            tc.tile_pool(name="outsb", bufs=2) as opool,
        ):
            # ---- load inputs (one packed transfer each, priority order) ----
            def load(name, dram, width):
                t = apool.tile([P, KT4 * width], BF16, name=name, tag=name)
                nc.sync.dma_start(t[:], dram[:, :])
                return t

            wv_t = load("wv", wv, D)
            ytA_t = load("yta", ytA, HKB)
            ytB_t = load("ytb", ytB, HKB)
            wk_t = load("wk", wk, D)
            wq_t = load("wq", wq, D)
            xT_t = load("xt", xT, SQ)
            wo_t = load("wo", wo, D)
            wk_sb = [wk_t[:, k * D : (k + 1) * D] for k in range(KT4)]
            wq_sb = [wq_t[:, k * D : (k + 1) * D] for k in range(KT4)]
            wv_sb = [wv_t[:, k * D : (k + 1) * D] for k in range(KT4)]
            wo_sb = [wo_t[:, k * D : (k + 1) * D] for k in range(KT4)]
            xT_sb = [xT_t[:, k * SQ : (k + 1) * SQ] for k in range(KT4)]

            def yt_cols(k, c0, c1):
                if c1 <= HKB:
                    return ytA_t[:, k * HKB + c0 : k * HKB + c1]
                assert c0 >= HKB
                return ytB_t[:, k * HKB + c0 - HKB : k * HKB + c1 - HKB]

            ones_sb = apool.tile([P, H], F32, name="ones_sb", tag="ones", bufs=1)
            nc.vector.memset(ones_sb[:], 1.0)
            ones_v = ones_sb.rearrange("p (h c) -> p h c", h=H, c=1)

            # HAM warm-up on the first-arriving tensor: the dummy burst runs
            # during the remaining DMA wait so the PE clock is at 2.4 GHz
            # (not the cold 1.2 GHz default) when the projections start.
            warm_ps = pspool.tile([P, SQ], F32, name="warmps", tag="lg", bufs=2)
            for _ in range(8):
                _mm(nc, warm_ps[:, :512], wv_t[:, :P], wv_t[:, :512], True, True)

            # ---- V projection first: attention needs all of V, while
            # KT[p]/QT[p] are only needed when head pair p starts.
            # V_aug[kt] = [128 keys, 8 heads * 65]; col 64 of each head = 1.0
            V_sb = []
            for kt in range(NKT):
                t = apool.tile([P, VAUG_W], BF16, name=f"vaug{kt}", tag=f"vaug{kt}")
                ps = pspool.tile(
                    [P, SQ],
                    F32,
                    name=f"vps{kt}",
                    tag=("at" if kt % 2 == 0 else "lg"),
                    bufs=2,
                )
                for k in range(KT4):
                    _mm(
                        nc,
                        ps[:, :512],
                        yt_cols(k, kt * P, (kt + 1) * P),
                        wv_sb[k][:],
                        start=(k == 0),
                        stop=(k == KT4 - 1),
                    )
                tv = t.rearrange("p (h c) -> p h c", h=H, c=DEPTH + 1)
                nc.vector.tensor_copy(
                    tv[:, :, 0:DEPTH],
                    ps[:, :512].rearrange("p (h c) -> p h c", h=H, c=DEPTH),
                )
                nc.vector.tensor_copy(tv[:, :, DEPTH : DEPTH + 1], ones_v)
                V_sb.append(t)

            # KT[p] = [128 outdims, 2048 keys]; QT[p] = [128 outdims, 1024 q].
            # Emitted per head pair: pairs 0/1 up front, later pairs at the
            # previous pair's boundary (overlaps the attention).
            QT_sb = [None] * KT4
            KT_sb = [None] * KT4

            def emit_kt_half(p, kb):
                if KT_sb[p] is None:
                    KT_sb[p] = apool.tile(
                        [P, SK], BF16, name=f"ktsb{p}", tag=f"ktsb{p}"
                    )
                t = KT_sb[p]
                ps = pspool.tile(
                    [P, SQ], F32, name=f"ktps{p}_{kb}", tag="lg", bufs=2
                )
                for qb in range(2):
                    for k in range(KT4):
                        _mm(
                            nc,
                            ps[:, qb * 512 : (qb + 1) * 512],
                            wk_sb[k][:, p * P : (p + 1) * P],
                            yt_cols(
                                k, kb * SQ + qb * 512, kb * SQ + (qb + 1) * 512
                            ),
                            start=(k == 0),
                            stop=(k == KT4 - 1),
                        )
                nc.vector.tensor_copy(t[:, kb * SQ : (kb + 1) * SQ], ps[:])

            def emit_qt(p):
                ps = pspool.tile([P, SQ], F32, name=f"qtps{p}", tag="lg", bufs=2)
                for qb in range(SQ // 512):
                    for k in range(KT4):
                        _mm(
                            nc,
                            ps[:, qb * 512 : (qb + 1) * 512],
                            wq_sb[k][:, p * P : (p + 1) * P],
                            xT_sb[k][:, qb * 512 : (qb + 1) * 512],
                            start=(k == 0),
                            stop=(k == KT4 - 1),
                        )
                t = apool.tile([P, SQ], BF16, name=f"qtsb{p}", tag=f"qtsb{p}")
                nc.vector.tensor_copy(t[:], ps[:])
                QT_sb[p] = t

            def emit_ktqt(p):
                emit_kt_half(p, 0)
                emit_kt_half(p, 1)
                emit_qt(p)

            # ---- attention, head-pair by head-pair ----
            attnT_sb = []
            for p in range(KT4):
                t = apool.tile([P, SQ], BF16, name=f"attnt{p}", tag=f"attnt{p}")
                attnT_sb.append(t)

            ops_pre = []
            emit_ktqt(0)
            emit_ktqt(1)
            for pr in range(KT4):
                attn_pair = []
                for half in range(2):
                    h = 2 * pr + half
                    t = pspool.tile(
                        [DEPTH + 1, SQ], F32, name=f"attnps{h}", tag="at", bufs=2
                    )
                    attn_pair.append(t)
                for kt in range(NKT):
                    for qb in range(2):
                        # logits: [128 keys, 1024] = [A qb-block | B qb-block];
                        # the two matmuls hit disjoint PE row groups -> run
                        # concurrently.
                        lg = pspool.tile(
                            [P, SQ], F32, name=f"lg{pr}_{kt}_{qb}", tag="lg", bufs=2
                        )
                        for half in range(2):
                            _mm(
                                nc,
                                lg[:, half * 512 : (half + 1) * 512],
                                KT_sb[pr][
                                    half * DEPTH : (half + 1) * DEPTH,
                                    kt * P : (kt + 1) * P,
                                ],
                                QT_sb[pr][
                                    half * DEPTH : (half + 1) * DEPTH,
                                    qb * 512 : (qb + 1) * 512,
                                ],
                                start=True,
                                stop=True,
                            )
                        pt = ptpool.tile(
                            [P, SQ], BF16, name=f"pt{pr}_{kt}_{qb}", tag="pt"
                        )
                        nc.scalar.activation(pt[:], lg[:], EXP)
                        for half in range(2):
                            h = 2 * pr + half
                            _mm(
                                nc,
                                attn_pair[half][:, qb * 512 : (qb + 1) * 512],
                                V_sb[kt][
                                    :, h * (DEPTH + 1) : (h + 1) * (DEPTH + 1)
                                ],
                                pt[:, half * 512 : (half + 1) * 512],
                                start=(kt == 0),
                                stop=(kt == NKT - 1),
                            )
                    if kt == 3 and pr + 2 < KT4:
                        emit_kt_half(pr + 2, 0)
                    elif kt == 7 and pr + 2 < KT4:
                        emit_kt_half(pr + 2, 1)
                    elif kt == 11 and pr + 2 < KT4:
                        emit_qt(pr + 2)
                if pr == KT4 - 1:
                    # Tail: pre-accumulate the k=0..2 output-projection
                    # chunks for the first two qt tiles (they depend only on
                    # pairs 0-2, which normalized long ago) and follow with a
                    # dummy burst sized to bridge the norm window, so the PE
                    # HAM clock stays at 2.4 GHz and the remaining out-proj
                    # matmuls run warm instead of at the cold 1.2 GHz.
                    for qt in range(2):
                        ops_pre.append(
                            pspool.tile(
                                [P, SQ], F32, name=f"ops{qt}", tag="lg", bufs=2
                            )
                        )
                        for k in range(KT4 - 1):
                            _mm(
                                nc,
                                ops_pre[qt][:, :512],
                                attnT_sb[k][:, qt * P : (qt + 1) * P],
                                wo_sb[k][:],
                                start=(k == 0),
                                stop=False,
                            )
                    warm_at = pspool.tile(
                        [DEPTH + 1, SQ], F32, name="warmat", tag="at", bufs=2
                    )
                    for _ in range(24):
                        _mm(nc, warm_at[:, :512], wo_t[:, : DEPTH + 1],
                            wo_t[:, :512], True, True)
                    # Latency-ordered normalize (only legal here: the psum
                    # release order no longer matters for a next pair).
                    # Denominator copies + reciprocals issue first so the
                    # gpsimd broadcasts start ~2.3us after the last attn
                    # matmul and hide under the bf16 evacuation casts.
                    recips3, auns3 = [], []
                    for half in range(2):
                        h = 2 * pr + half
                        den = spool.tile([1, SQ], F32, name=f"den{h}", tag="den")
                        nc.vector.tensor_copy(
                            den[:], attn_pair[half][DEPTH : DEPTH + 1, :]
                        )
                        recip = spool.tile(
                            [1, SQ], F32, name=f"recip{h}", tag="recip"
                        )
                        nc.vector.reciprocal_approx_fast(recip[:], den[:])
                        recips3.append(recip)
                        aun = spool.tile(
                            [DEPTH, SQ], BF16, name=f"aun{h}", tag="aun"
                        )
                        nc.vector.tensor_copy(aun[:], attn_pair[half][0:DEPTH, :])
                        auns3.append(aun)
                    for half in range(2):
                        h = 2 * pr + half
                        bcast = spool.tile(
                            [DEPTH, SQ], F32, name=f"bcast{h}", tag="bcast"
                        )
                        nc.gpsimd.partition_broadcast(bcast[:], recips3[half][:])
                        dst = attnT_sb[pr][half * DEPTH : (half + 1) * DEPTH, :]
                        nc.vector.tensor_mul(dst, auns3[half][:], bcast[:])
                    continue
                # Evacuate both heads' PSUM first: one [65, 1024] copy per
                # head releases the attn psum slots quickly so the next
                # pair's matmuls keep the PE busy. Then the denominator row
                # is copied to partition 0 (the custom DVE reciprocal reads
                # partition 0 only), reciprocal_approx_fast (~1.1us vs 6.5us
                # for the iterative reciprocal), gpsimd broadcast, multiply.
                auns = []
                for half in range(2):
                    h = 2 * pr + half
                    aun = spool.tile([DEPTH + 1, SQ], F32, name=f"aun{h}", tag="aun")
                    nc.vector.tensor_copy(aun[:], attn_pair[half][0 : DEPTH + 1, :])
                    auns.append(aun)
                recips = []
                for half in range(2):
                    h = 2 * pr + half
                    den = spool.tile([1, SQ], F32, name=f"den{h}", tag="den")
                    nc.vector.tensor_copy(den[:], auns[half][DEPTH : DEPTH + 1, :])
                    recip = spool.tile([1, SQ], F32, name=f"recip{h}", tag="recip")
                    nc.vector.reciprocal_approx_fast(recip[:], den[:])
                    recips.append(recip)
                for half in range(2):
                    h = 2 * pr + half
                    dst = attnT_sb[pr][half * DEPTH : (half + 1) * DEPTH, :]
                    bcast = spool.tile(
                        [DEPTH, SQ], F32, name=f"bcast{h}", tag="bcast"
                    )
                    nc.gpsimd.partition_broadcast(bcast[:], recips[half][:])
                    nc.vector.tensor_mul(dst, auns[half][0:DEPTH, :], bcast[:])

            # ---- output projection: out[q, od] = attnT^T @ Wo ----
            for qt in range(NQT):
                if qt < 2:
                    ps = ops_pre[qt]
                    _mm(
                        nc,
                        ps[:, :512],
                        attnT_sb[KT4 - 1][:, qt * P : (qt + 1) * P],
                        wo_sb[KT4 - 1][:],
                        start=False,
                        stop=True,
                    )
                else:
                    ps = pspool.tile(
                        [P, SQ], F32, name=f"ops{qt}", tag="lg", bufs=2
                    )
                    for k in range(KT4):
                        _mm(
                            nc,
                            ps[:, :512],
                            attnT_sb[k][:, qt * P : (qt + 1) * P],
                            wo_sb[k][:],
                            start=(k == 0),
                            stop=(k == KT4 - 1),
                        )
                osb = opool.tile([P, D], BF16, name=f"osb{qt}", tag="osb")
                nc.vector.tensor_copy(osb[:], ps[:, :512])
                nc.sync.dma_start(out[qt * P : (qt + 1) * P, :], osb[:])

    nc.compile()
    return nc


_CACHE: dict = {}


def get_nc():
    if "nc" not in _CACHE:
        _CACHE["nc"] = build_nc()
    return _CACHE["nc"]


def _pack(a):
    """[512, X] -> [128, 4*X]: k-tile k lands at columns [k*X, (k+1)*X)."""
    d, x = a.shape
    assert d == D
    return np.ascontiguousarray(
        a.reshape(KT4, P, x).transpose(1, 0, 2).reshape(P, KT4 * x)
    )


def make_in_maps(x, y, W_q, W_k, W_v, W_o):
    bf = ml_dtypes.bfloat16
    x = np.ascontiguousarray(x, dtype=np.float32)
    y = np.ascontiguousarray(y, dtype=np.float32)
    wq = _pack((np.asarray(W_q, np.float32) * np.float32(DEPTH**-0.5)).astype(bf))
    wk = _pack(np.asarray(W_k, dtype=np.float32).astype(bf))
    wv = _pack(np.asarray(W_v, dtype=np.float32).astype(bf))
    wo = _pack(np.asarray(W_o, dtype=np.float32).astype(bf))
    in_maps = []
    for b in range(B):
        yT = y[b].T.astype(bf)
        ytA = _pack(yT[:, :HKB])
        ytB = _pack(yT[:, HKB:])
        for half in range(2):
            in_maps.append(
                {
                    "xT": _pack(x[b, half * SQ : (half + 1) * SQ, :].T.astype(bf)),
                    "ytA": ytA,
                    "ytB": ytB,
                    "wq": wq,
                    "wk": wk,
                    "wv": wv,
                    "wo": wo,
                }
            )
    return in_maps


def assemble_out(results):
    out = np.empty((B, S, D), np.float32)
    for c in range(N_CORES):
        b, half = c // 2, c % 2
        out[b, half * SQ : (half + 1) * SQ, :] = results[c]["out"].astype(
            np.float32
        )
    return out


def kernel(x, y, W_q, W_k, W_v, W_o):
    nc = get_nc()
    in_maps = make_in_maps(x, y, W_q, W_k, W_v, W_o)
    res = run_bass_kernel_spmd(nc, in_maps, core_ids=list(range(N_CORES)))
    return assemble_out(res.results)


# revision 18
# speedup vs baseline: 1.0298x; 1.0298x over previous
"""Multi-head attention (B=4, S=2048, D=512, H=8) on 8 Trainium2 cores.

Sharding: core c = (batch b = c//2, query-half = c%2). Each core computes
1024 query rows of one batch over all 2048 keys and all 8 heads, producing
a disjoint slice of the output -> no inter-core reduction needed.

Per-core layout is fully "transposed land" (contraction dim on partitions):
  xT [512,1024], yT [512,2048] prepared (transposed, bf16, k-tile-packed)
  on host.
  QT = Wq^T @ xT   (Wq pre-scaled by depth^-0.5 on host)
  KT = Wk^T @ yT
  V  = y @ Wv in natural [keys, dim] layout, stored strided into
       V_aug [128, 8*65] with a ones column per head (row 64 of the
       attention matmul output then accumulates softmax denominators).
  per head pair (2p, 2p+1): head A lives on partitions 0:64, head B on
       64:128 of the shared KT/QT tile, so their logits matmuls target
       disjoint PE row groups and run concurrently.
       logitsT[kt] = (KT tile)^T @ QT  (bf16 operands, fp32 PSUM)
       -> one exp over [128, 1024] (ScalarE, PSUM -> SBUF bf16)
       -> attnT += V_aug^T @ PT, fp32 PSUM, accumulated over 16 key tiles.
  normalize: evacuate PSUM fast (DVE copies release the accumulation
       banks so the next pair keeps the PE busy); the denominator row is
       copied to partition 0 of a small tile where the partition-0-only
       custom DVE reciprocal_approx_fast (~51 ULP, ~5x faster than the
       iterative reciprocal) reads it; gpsimd partition_broadcast and the
       multiply run off the critical path.
  out = attnT^T @ Wo -> DMA (fp32).

DMA: every input arrives pre-packed as [128, k-tiles * cols] so each
tensor is ONE transfer with 2-8KB contiguous rows sprayed across all 16
queues (1KB rows cost ~4x more in descriptor overhead), emitted in
consumption order (wv, yT, wk, wq, xT, wo). A ~3.4us burst of dummy
matmuls on the first-arriving tensor warms the PE HAM clock to 2.4 GHz
before the real projections start.

Softmax skips max-subtraction (logits ~ N(0,1); exp cannot overflow fp32).
End-to-end RMS relative error vs fp32 ~5e-3.
"""

import numpy as np
import ml_dtypes

import concourse.bass as bass
import concourse.tile as tile
from concourse import bacc, mybir
from concourse.bass_utils import run_bass_kernel_spmd

F32 = mybir.dt.float32
BF16 = mybir.dt.bfloat16
EXP = mybir.ActivationFunctionType.Exp

B, S, D = 4, 2048, 512
H = 8
DEPTH = D // H  # 64
SQ = S // 2  # queries per core (1024)
SK = S  # keys per core (2048)
N_CORES = 8

P = 128
KT4 = D // P  # 4 contraction tiles for projections
NKT = SK // P  # 16 key tiles
NQT = SQ // P  # 8 query tiles
VAUG_W = H * (DEPTH + 1)  # 520
HKB = SK // 2  # 1024 keys per yT half


def _mm(nc, out, lhsT, rhs, start, stop):
    nc.tensor.matmul(out, lhsT, rhs, start=start, stop=stop)


def build_nc():
    nc = bacc.Bacc("TRN2", target_bir_lowering=False, debug=False)

    xT = nc.dram_tensor("xT", [P, KT4 * SQ], BF16, kind="ExternalInput").ap()
    ytA = nc.dram_tensor("ytA", [P, KT4 * HKB], BF16, kind="ExternalInput").ap()
    ytB = nc.dram_tensor("ytB", [P, KT4 * HKB], BF16, kind="ExternalInput").ap()
    wq = nc.dram_tensor("wq", [P, KT4 * D], BF16, kind="ExternalInput").ap()
    wk = nc.dram_tensor("wk", [P, KT4 * D], BF16, kind="ExternalInput").ap()
    wv = nc.dram_tensor("wv", [P, KT4 * D], BF16, kind="ExternalInput").ap()
    wo = nc.dram_tensor("wo", [P, KT4 * D], BF16, kind="ExternalInput").ap()
    # Output is shipped bf16 (half the DMA bytes of the fp32 result; the
    # host casts back to fp32 -- adds ~0.2% RMS, well inside the budget).
    out = nc.dram_tensor("out", [SQ, D], BF16, kind="ExternalOutput").ap()

    with tile.TileContext(nc) as tc:
        with (
            tc.tile_pool(name="acts", bufs=1) as apool,
            tc.tile_pool(name="ps", bufs=1, space="PSUM") as pspool,
            tc.tile_pool(name="pt", bufs=8) as ptpool,
            tc.tile_pool(name="small", bufs=2) as spool,
            tc.tile_pool(name="outsb", bufs=4) as opool,
        ):
            # ---- load inputs (one packed transfer each, priority order) ----
            def load(name, dram, width):
                t = apool.tile([P, KT4 * width], BF16, name=name, tag=name)
                nc.sync.dma_start(t[:], dram[:, :])
                return t

            wv_t = load("wv", wv, D)
            ytA_t = load("yta", ytA, HKB)
            ytB_t = load("ytb", ytB, HKB)
            wk_t = load("wk", wk, D)
            wq_t = load("wq", wq, D)
            xT_t = load("xt", xT, SQ)
            wo_t = load("wo", wo, D)
            wk_sb = [wk_t[:, k * D : (k + 1) * D] for k in range(KT4)]
            wq_sb = [wq_t[:, k * D : (k + 1) * D] for k in range(KT4)]
            wv_sb = [wv_t[:, k * D : (k + 1) * D] for k in range(KT4)]
            wo_sb = [wo_t[:, k * D : (k + 1) * D] for k in range(KT4)]
            xT_sb = [xT_t[:, k * SQ : (k + 1) * SQ] for k in range(KT4)]

            def yt_cols(k, c0, c1):
                if c1 <= HKB:
                    return ytA_t[:, k * HKB + c0 : k * HKB + c1]
                assert c0 >= HKB
                return ytB_t[:, k * HKB + c0 - HKB : k * HKB + c1 - HKB]

            ones_sb = apool.tile([P, H], F32, name="ones_sb", tag="ones", bufs=1)
            nc.vector.memset(ones_sb[:], 1.0)
            ones_v = ones_sb.rearrange("p (h c) -> p h c", h=H, c=1)

            # HAM warm-up on the first-arriving tensor: the dummy burst runs
            # during the remaining DMA wait so the PE clock is at 2.4 GHz
            # (not the cold 1.2 GHz default) when the projections start.
            warm_ps = pspool.tile([P, SQ], F32, name="warmps", tag="lg", bufs=2)
            for _ in range(8):
                _mm(nc, warm_ps[:, :512], wv_t[:, :P], wv_t[:, :512], True, True)

            # ---- V projection first: attention needs all of V, while
            # KT[p]/QT[p] are only needed when head pair p starts.
            # V_aug[kt] = [128 keys, 8 heads * 65]; col 64 of each head = 1.0
            V_sb = []
            for kt in range(NKT):
                t = apool.tile([P, VAUG_W], BF16, name=f"vaug{kt}", tag=f"vaug{kt}")
                ps = pspool.tile(
                    [P, SQ],
                    F32,
                    name=f"vps{kt}",
                    tag=("at" if kt % 2 == 0 else "lg"),
                    bufs=2,
                )
                for k in range(KT4):
                    _mm(
                        nc,
                        ps[:, :512],
                        yt_cols(k, kt * P, (kt + 1) * P),
                        wv_sb[k][:],
                        start=(k == 0),
                        stop=(k == KT4 - 1),
                    )
                tv = t.rearrange("p (h c) -> p h c", h=H, c=DEPTH + 1)
                nc.vector.tensor_copy(
                    tv[:, :, 0:DEPTH],
                    ps[:, :512].rearrange("p (h c) -> p h c", h=H, c=DEPTH),
                )
                nc.vector.tensor_copy(tv[:, :, DEPTH : DEPTH + 1], ones_v)
                V_sb.append(t)

            # KT[p] = [128 outdims, 2048 keys]; QT[p] = [128 outdims, 1024 q].
            # Emitted per head pair: pairs 0/1 up front, later pairs at the
            # previous pair's boundary (overlaps the attention).
            QT_sb = [None] * KT4
            KT_sb = [None] * KT4

            def emit_kt_half(p, kb):
                if KT_sb[p] is None:
                    KT_sb[p] = apool.tile(
                        [P, SK], BF16, name=f"ktsb{p}", tag=f"ktsb{p}"
                    )
                t = KT_sb[p]
                ps = pspool.tile(
                    [P, SQ], F32, name=f"ktps{p}_{kb}", tag="lg", bufs=2
                )
                for qb in range(2):
                    for k in range(KT4):
                        _mm(
                            nc,
                            ps[:, qb * 512 : (qb + 1) * 512],
                            wk_sb[k][:, p * P : (p + 1) * P],
                            yt_cols(
                                k, kb * SQ + qb * 512, kb * SQ + (qb + 1) * 512
                            ),
                            start=(k == 0),
                            stop=(k == KT4 - 1),
                        )
                nc.vector.tensor_copy(t[:, kb * SQ : (kb + 1) * SQ], ps[:])

            def emit_qt(p):
                ps = pspool.tile([P, SQ], F32, name=f"qtps{p}", tag="lg", bufs=2)
                for qb in range(SQ // 512):
                    for k in range(KT4):
                        _mm(
                            nc,
                            ps[:, qb * 512 : (qb + 1) * 512],
                            wq_sb[k][:, p * P : (p + 1) * P],
                            xT_sb[k][:, qb * 512 : (qb + 1) * 512],
                            start=(k == 0),
                            stop=(k == KT4 - 1),
                        )
                t = apool.tile([P, SQ], BF16, name=f"qtsb{p}", tag=f"qtsb{p}")
                nc.vector.tensor_copy(t[:], ps[:])
                QT_sb[p] = t

            def emit_ktqt(p):
                emit_kt_half(p, 0)
                emit_kt_half(p, 1)
                emit_qt(p)

            # ---- attention, head-pair by head-pair ----
            attnT_sb = []
            for p in range(KT4):
                t = apool.tile([P, SQ], BF16, name=f"attnt{p}", tag=f"attnt{p}")
                attnT_sb.append(t)

            emit_ktqt(0)
            emit_ktqt(1)
            for pr in range(KT4):
                attn_pair = []
                for half in range(2):
                    h = 2 * pr + half
                    t = pspool.tile(
                        [DEPTH + 1, SQ], F32, name=f"attnps{h}", tag="at", bufs=2
                    )
                    attn_pair.append(t)
                for kt in range(NKT):
                    for qb in range(2):
                        # logits: [128 keys, 1024] = [A qb-block | B qb-block];
                        # the two matmuls hit disjoint PE row groups -> run
                        # concurrently.
                        lg = pspool.tile(
                            [P, SQ], F32, name=f"lg{pr}_{kt}_{qb}", tag="lg", bufs=2
                        )
                        for half in range(2):
                            _mm(
                                nc,
                                lg[:, half * 512 : (half + 1) * 512],
                                KT_sb[pr][
                                    half * DEPTH : (half + 1) * DEPTH,
                                    kt * P : (kt + 1) * P,
                                ],
                                QT_sb[pr][
                                    half * DEPTH : (half + 1) * DEPTH,
                                    qb * 512 : (qb + 1) * 512,
                                ],
                                start=True,
                                stop=True,
                            )
                        pt = ptpool.tile(
                            [P, SQ], BF16, name=f"pt{pr}_{kt}_{qb}", tag="pt"
                        )
                        nc.scalar.activation(pt[:], lg[:], EXP)
                        for half in range(2):
                            h = 2 * pr + half
                            _mm(
                                nc,
                                attn_pair[half][:, qb * 512 : (qb + 1) * 512],
                                V_sb[kt][
                                    :, h * (DEPTH + 1) : (h + 1) * (DEPTH + 1)
                                ],
                                pt[:, half * 512 : (half + 1) * 512],
                                start=(kt == 0),
                                stop=(kt == NKT - 1),
                            )
                    if kt == 3 and pr + 2 < KT4:
                        emit_kt_half(pr + 2, 0)
                    elif kt == 7 and pr + 2 < KT4:
                        emit_kt_half(pr + 2, 1)
                    elif kt == 11 and pr + 2 < KT4:
                        emit_qt(pr + 2)
                # Evacuate both heads' PSUM first: one [65, 1024] copy per
                # head releases the attn psum slots quickly so the next
                # pair's matmuls keep the PE busy. Then the denominator row
                # is copied to partition 0 (the custom DVE reciprocal reads
                # partition 0 only), reciprocal_approx_fast (~1.1us vs 6.5us
                # for the iterative reciprocal), gpsimd broadcast, multiply.
                auns = []
                for half in range(2):
                    h = 2 * pr + half
                    aun = spool.tile([DEPTH + 1, SQ], F32, name=f"aun{h}", tag="aun")
                    nc.vector.tensor_copy(aun[:], attn_pair[half][0 : DEPTH + 1, :])
                    auns.append(aun)
                recips = []
                for half in range(2):
                    h = 2 * pr + half
                    den = spool.tile([1, SQ], F32, name=f"den{h}", tag="den")
                    nc.vector.tensor_copy(den[:], auns[half][DEPTH : DEPTH + 1, :])
                    recip = spool.tile([1, SQ], F32, name=f"recip{h}", tag="recip")
                    nc.vector.reciprocal_approx_fast(recip[:], den[:])
                    recips.append(recip)
                for half in range(2):
                    h = 2 * pr + half
                    dst = attnT_sb[pr][half * DEPTH : (half + 1) * DEPTH, :]
                    bcast = spool.tile(
                        [DEPTH, SQ], F32, name=f"bcast{h}", tag="bcast"
                    )
                    nc.gpsimd.partition_broadcast(bcast[:], recips[half][:])
                    nc.vector.tensor_mul(dst, auns[half][0:DEPTH, :], bcast[:])

            # ---- output projection: out[q, od] = attnT^T @ Wo ----
            for qt in range(NQT):
                ps = pspool.tile([P, SQ], F32, name=f"ops{qt}", tag="lg", bufs=2)
                for k in range(KT4):
                    _mm(
                        nc,
                        ps[:, :512],
                        attnT_sb[k][:, qt * P : (qt + 1) * P],
                        wo_sb[k][:],
                        start=(k == 0),
                        stop=(k == KT4 - 1),
                    )
                osb = opool.tile([P, D], BF16, name=f"osb{qt}", tag="osb")
                nc.vector.tensor_copy(osb[:], ps[:, :512])
                nc.sync.dma_start(out[qt * P : (qt + 1) * P, :], osb[:])

    nc.compile()
    return nc


_CACHE: dict = {}


def get_nc():
    if "nc" not in _CACHE:
        _CACHE["nc"] = build_nc()
    return _CACHE["nc"]


def _pack(a):
    """[512, X] -> [128, 4*X]: k-tile k lands at columns [k*X, (k+1)*X)."""
    d, x = a.shape
    assert d == D
    return np.ascontiguousarray(
        a.reshape(KT4, P, x).transpose(1, 0, 2).reshape(P, KT4 * x)
    )


def make_in_maps(x, y, W_q, W_k, W_v, W_o):
    bf = ml_dtypes.bfloat16
    x = np.ascontiguousarray(x, dtype=np.float32)
    y = np.ascontiguousarray(y, dtype=np.float32)
    wq = _pack((np.asarray(W_q, np.float32) * np.float32(DEPTH**-0.5)).astype(bf))
    wk = _pack(np.asarray(W_k, dtype=np.float32).astype(bf))
    wv = _pack(np.asarray(W_v, dtype=np.float32).astype(bf))
    wo = _pack(np.asarray(W_o, dtype=np.float32).astype(bf))
    in_maps = []
    for b in range(B):
        yT = y[b].T.astype(bf)
        ytA = _pack(yT[:, :HKB])
        ytB = _pack(yT[:, HKB:])
        for half in range(2):
            in_maps.append(
                {
                    "xT": _pack(x[b, half * SQ : (half + 1) * SQ, :].T.astype(bf)),
                    "ytA": ytA,
                    "ytB": ytB,
                    "wq": wq,
                    "wk": wk,
                    "wv": wv,
                    "wo": wo,
                }
            )
    return in_maps


def assemble_out(results):
    out = np.empty((B, S, D), np.float32)
    for c in range(N_CORES):
        b, half = c // 2, c % 2
        out[b, half * SQ : (half + 1) * SQ, :] = results[c]["out"].astype(
            np.float32
        )
    return out


def kernel(x, y, W_q, W_k, W_v, W_o):
    nc = get_nc()
    in_maps = make_in_maps(x, y, W_q, W_k, W_v, W_o)
    res = run_bass_kernel_spmd(nc, in_maps, core_ids=list(range(N_CORES)))
    return assemble_out(res.results)


# revision 19
# speedup vs baseline: 1.0302x; 1.0003x over previous
"""Multi-head attention (B=4, S=2048, D=512, H=8) on 8 Trainium2 cores.

Sharding: core c = (batch b = c//2, query-half = c%2). Each core computes
1024 query rows of one batch over all 2048 keys and all 8 heads, producing
a disjoint slice of the output -> no inter-core reduction needed.

Per-core layout is fully "transposed land" (contraction dim on partitions):
  xT [512,1024], yT [512,2048] prepared (transposed, bf16, k-tile-packed)
  on host.
  QT = Wq^T @ xT   (Wq pre-scaled by depth^-0.5 on host)
  KT = Wk^T @ yT
  V  = y @ Wv in natural [keys, dim] layout, stored strided into
       V_aug [128, 8*65] with a ones column per head (row 64 of the
       attention matmul output then accumulates softmax denominators).
  per head pair (2p, 2p+1): head A lives on partitions 0:64, head B on
       64:128 of the shared KT/QT tile, so their logits matmuls target
       disjoint PE row groups and run concurrently.
       logitsT[kt] = (KT tile)^T @ QT  (bf16 operands, fp32 PSUM)
       -> one exp over [128, 1024] (ScalarE, PSUM -> SBUF bf16)
       -> attnT += V_aug^T @ PT, fp32 PSUM, accumulated over 16 key tiles.
  normalize: evacuate PSUM fast (DVE copies release the accumulation
       banks so the next pair keeps the PE busy); the denominator row is
       copied to partition 0 of a small tile where the partition-0-only
       custom DVE reciprocal_approx_fast (~51 ULP, ~5x faster than the
       iterative reciprocal) reads it; gpsimd partition_broadcast and the
       multiply run off the critical path.
  out = attnT^T @ Wo -> DMA (fp32).

DMA: every input arrives pre-packed as [128, k-tiles * cols] so each
tensor is ONE transfer with 2-8KB contiguous rows sprayed across all 16
queues (1KB rows cost ~4x more in descriptor overhead), emitted in
consumption order (wv, yT, wk, wq, xT, wo). A ~3.4us burst of dummy
matmuls on the first-arriving tensor warms the PE HAM clock to 2.4 GHz
before the real projections start.

Softmax skips max-subtraction (logits ~ N(0,1); exp cannot overflow fp32).
End-to-end RMS relative error vs fp32 ~5e-3.
"""

import numpy as np
import ml_dtypes

import concourse.bass as bass
import concourse.tile as tile
from concourse import bacc, mybir
from concourse.bass_utils import run_bass_kernel_spmd

F32 = mybir.dt.float32
BF16 = mybir.dt.bfloat16
EXP = mybir.ActivationFunctionType.Exp

B, S, D = 4, 2048, 512
H = 8
DEPTH = D // H  # 64
SQ = S // 2  # queries per core (1024)
SK = S  # keys per core (2048)
N_CORES = 8

P = 128
KT4 = D // P  # 4 contraction tiles for projections
NKT = SK // P  # 16 key tiles
NQT = SQ // P  # 8 query tiles
VAUG_W = H * (DEPTH + 1)  # 520
HKB = SK // 2  # 1024 keys per yT half


def _mm(nc, out, lhsT, rhs, start, stop):
    nc.tensor.matmul(out, lhsT, rhs, start=start, stop=stop)


def build_nc():
    nc = bacc.Bacc("TRN2", target_bir_lowering=False, debug=False)

    xT = nc.dram_tensor("xT", [P, KT4 * SQ], BF16, kind="ExternalInput").ap()
    ytA = nc.dram_tensor("ytA", [P, KT4 * HKB], BF16, kind="ExternalInput").ap()
    ytB = nc.dram_tensor("ytB", [P, KT4 * HKB], BF16, kind="ExternalInput").ap()
    wq = nc.dram_tensor("wq", [P, KT4 * D], BF16, kind="ExternalInput").ap()
    wk = nc.dram_tensor("wk", [P, KT4 * D], BF16, kind="ExternalInput").ap()
    wv = nc.dram_tensor("wv", [P, KT4 * D], BF16, kind="ExternalInput").ap()
    wo = nc.dram_tensor("wo", [P, KT4 * D], BF16, kind="ExternalInput").ap()
    # Output is shipped bf16 (half the DMA bytes of the fp32 result; the
    # host casts back to fp32 -- adds ~0.2% RMS, well inside the budget).
    out = nc.dram_tensor("out", [SQ, D], BF16, kind="ExternalOutput").ap()

    with tile.TileContext(nc) as tc:
        with (
            tc.tile_pool(name="acts", bufs=1) as apool,
            tc.tile_pool(name="ps", bufs=1, space="PSUM") as pspool,
            tc.tile_pool(name="pt", bufs=10) as ptpool,
            tc.tile_pool(name="small", bufs=3) as spool,
            tc.tile_pool(name="outsb", bufs=6) as opool,
        ):
            # ---- load inputs (one packed transfer each, priority order) ----
            def load(name, dram, width):
                t = apool.tile([P, KT4 * width], BF16, name=name, tag=name)
                nc.sync.dma_start(t[:], dram[:, :])
                return t

            wv_t = load("wv", wv, D)
            ytA_t = load("yta", ytA, HKB)
            ytB_t = load("ytb", ytB, HKB)
            wk_t = load("wk", wk, D)
            wq_t = load("wq", wq, D)
            xT_t = load("xt", xT, SQ)
            wo_t = load("wo", wo, D)
            wk_sb = [wk_t[:, k * D : (k + 1) * D] for k in range(KT4)]
            wq_sb = [wq_t[:, k * D : (k + 1) * D] for k in range(KT4)]
            wv_sb = [wv_t[:, k * D : (k + 1) * D] for k in range(KT4)]
            wo_sb = [wo_t[:, k * D : (k + 1) * D] for k in range(KT4)]
            xT_sb = [xT_t[:, k * SQ : (k + 1) * SQ] for k in range(KT4)]

            def yt_cols(k, c0, c1):
                if c1 <= HKB:
                    return ytA_t[:, k * HKB + c0 : k * HKB + c1]
                assert c0 >= HKB
                return ytB_t[:, k * HKB + c0 - HKB : k * HKB + c1 - HKB]

            ones_sb = apool.tile([P, H], F32, name="ones_sb", tag="ones", bufs=1)
            nc.vector.memset(ones_sb[:], 1.0)
            ones_v = ones_sb.rearrange("p (h c) -> p h c", h=H, c=1)

            # HAM warm-up on the first-arriving tensor: the dummy burst runs
            # during the remaining DMA wait so the PE clock is at 2.4 GHz
            # (not the cold 1.2 GHz default) when the projections start.
            warm_ps = pspool.tile([P, SQ], F32, name="warmps", tag="lg", bufs=2)
            for _ in range(8):
                _mm(nc, warm_ps[:, :512], wv_t[:, :P], wv_t[:, :512], True, True)

            # ---- V projection first: attention needs all of V, while
            # KT[p]/QT[p] are only needed when head pair p starts.
            # V_aug[kt] = [128 keys, 8 heads * 65]; col 64 of each head = 1.0
            V_sb = []
            for kt in range(NKT):
                t = apool.tile([P, VAUG_W], BF16, name=f"vaug{kt}", tag=f"vaug{kt}")
                ps = pspool.tile(
                    [P, SQ],
                    F32,
                    name=f"vps{kt}",
                    tag=("at" if kt % 2 == 0 else "lg"),
                    bufs=2,
                )
                for k in range(KT4):
                    _mm(
                        nc,
                        ps[:, :512],
                        yt_cols(k, kt * P, (kt + 1) * P),
                        wv_sb[k][:],
                        start=(k == 0),
                        stop=(k == KT4 - 1),
                    )
                tv = t.rearrange("p (h c) -> p h c", h=H, c=DEPTH + 1)
                nc.vector.tensor_copy(
                    tv[:, :, 0:DEPTH],
                    ps[:, :512].rearrange("p (h c) -> p h c", h=H, c=DEPTH),
                )
                nc.vector.tensor_copy(tv[:, :, DEPTH : DEPTH + 1], ones_v)
                V_sb.append(t)

            # KT[p] = [128 outdims, 2048 keys]; QT[p] = [128 outdims, 1024 q].
            # Emitted per head pair: pairs 0/1 up front, later pairs at the
            # previous pair's boundary (overlaps the attention).
            QT_sb = [None] * KT4
            KT_sb = [None] * KT4

            def emit_kt_half(p, kb):
                if KT_sb[p] is None:
                    KT_sb[p] = apool.tile(
                        [P, SK], BF16, name=f"ktsb{p}", tag=f"ktsb{p}"
                    )
                t = KT_sb[p]
                ps = pspool.tile(
                    [P, SQ], F32, name=f"ktps{p}_{kb}", tag="lg", bufs=2
                )
                for qb in range(2):
                    for k in range(KT4):
                        _mm(
                            nc,
                            ps[:, qb * 512 : (qb + 1) * 512],
                            wk_sb[k][:, p * P : (p + 1) * P],
                            yt_cols(
                                k, kb * SQ + qb * 512, kb * SQ + (qb + 1) * 512
                            ),
                            start=(k == 0),
                            stop=(k == KT4 - 1),
                        )
                nc.vector.tensor_copy(t[:, kb * SQ : (kb + 1) * SQ], ps[:])

            def emit_qt(p):
                ps = pspool.tile([P, SQ], F32, name=f"qtps{p}", tag="lg", bufs=2)
                for qb in range(SQ // 512):
                    for k in range(KT4):
                        _mm(
                            nc,
                            ps[:, qb * 512 : (qb + 1) * 512],
                            wq_sb[k][:, p * P : (p + 1) * P],
                            xT_sb[k][:, qb * 512 : (qb + 1) * 512],
                            start=(k == 0),
                            stop=(k == KT4 - 1),
                        )
                t = apool.tile([P, SQ], BF16, name=f"qtsb{p}", tag=f"qtsb{p}")
                nc.vector.tensor_copy(t[:], ps[:])
                QT_sb[p] = t

            def emit_ktqt(p):
                emit_kt_half(p, 0)
                emit_kt_half(p, 1)
                emit_qt(p)

            # ---- attention, head-pair by head-pair ----
            attnT_sb = []
            for p in range(KT4):
                t = apool.tile([P, SQ], BF16, name=f"attnt{p}", tag=f"attnt{p}")
                attnT_sb.append(t)

            emit_ktqt(0)
            emit_ktqt(1)
            for pr in range(KT4):
                attn_pair = []
                for half in range(2):
                    h = 2 * pr + half
                    t = pspool.tile(
                        [DEPTH + 1, SQ], F32, name=f"attnps{h}", tag="at", bufs=2
                    )
                    attn_pair.append(t)
                for kt in range(NKT):
                    for qb in range(2):
                        # logits: [128 keys, 1024] = [A qb-block | B qb-block];
                        # the two matmuls hit disjoint PE row groups -> run
                        # concurrently.
                        lg = pspool.tile(
                            [P, SQ], F32, name=f"lg{pr}_{kt}_{qb}", tag="lg", bufs=2
                        )
                        for half in range(2):
                            _mm(
                                nc,
                                lg[:, half * 512 : (half + 1) * 512],
                                KT_sb[pr][
                                    half * DEPTH : (half + 1) * DEPTH,
                                    kt * P : (kt + 1) * P,
                                ],
                                QT_sb[pr][
                                    half * DEPTH : (half + 1) * DEPTH,
                                    qb * 512 : (qb + 1) * 512,
                                ],
                                start=True,
                                stop=True,
                            )
                        pt = ptpool.tile(
                            [P, SQ], BF16, name=f"pt{pr}_{kt}_{qb}", tag="pt"
                        )
                        nc.scalar.activation(pt[:], lg[:], EXP)
                        for half in range(2):
                            h = 2 * pr + half
                            _mm(
                                nc,
                                attn_pair[half][:, qb * 512 : (qb + 1) * 512],
                                V_sb[kt][
                                    :, h * (DEPTH + 1) : (h + 1) * (DEPTH + 1)
                                ],
                                pt[:, half * 512 : (half + 1) * 512],
                                start=(kt == 0),
                                stop=(kt == NKT - 1),
                            )
                    if kt == 3 and pr + 2 < KT4:
                        emit_kt_half(pr + 2, 0)
                    elif kt == 7 and pr + 2 < KT4:
                        emit_kt_half(pr + 2, 1)
                    elif kt == 11 and pr + 2 < KT4:
                        emit_qt(pr + 2)
                # Evacuate both heads' PSUM first: one [65, 1024] copy per
                # head releases the attn psum slots quickly so the next
                # pair's matmuls keep the PE busy. Then the denominator row
                # is copied to partition 0 (the custom DVE reciprocal reads
                # partition 0 only), reciprocal_approx_fast (~1.1us vs 6.5us
                # for the iterative reciprocal), gpsimd broadcast, multiply.
                auns = []
                for half in range(2):
                    h = 2 * pr + half
                    aun = spool.tile([DEPTH + 1, SQ], F32, name=f"aun{h}", tag="aun")
                    nc.vector.tensor_copy(aun[:], attn_pair[half][0 : DEPTH + 1, :])
                    auns.append(aun)
                recips = []
                for half in range(2):
                    h = 2 * pr + half
                    den = spool.tile([1, SQ], F32, name=f"den{h}", tag="den")
                    nc.vector.tensor_copy(den[:], auns[half][DEPTH : DEPTH + 1, :])
                    recip = spool.tile([1, SQ], F32, name=f"recip{h}", tag="recip")
                    nc.vector.reciprocal_approx_fast(recip[:], den[:])
                    recips.append(recip)
                for half in range(2):
                    h = 2 * pr + half
                    dst = attnT_sb[pr][half * DEPTH : (half + 1) * DEPTH, :]
                    bcast = spool.tile(
                        [DEPTH, SQ], F32, name=f"bcast{h}", tag="bcast"
                    )
                    nc.gpsimd.partition_broadcast(bcast[:], recips[half][:])
                    nc.vector.tensor_mul(dst, auns[half][0:DEPTH, :], bcast[:])

            # ---- output projection: out[q, od] = attnT^T @ Wo ----
            for qt in range(NQT):
                ps = pspool.tile([P, SQ], F32, name=f"ops{qt}", tag="lg", bufs=2)
                for k in range(KT4):
                    _mm(
                        nc,
                        ps[:, :512],
                        attnT_sb[k][:, qt * P : (qt + 1) * P],
                        wo_sb[k][:],
                        start=(k == 0),
                        stop=(k == KT4 - 1),
                    )
                osb = opool.tile([P, D], BF16, name=f"osb{qt}", tag="osb")
                nc.vector.tensor_copy(osb[:], ps[:, :512])
                nc.sync.dma_start(out[qt * P : (qt + 1) * P, :], osb[:])

    nc.compile()
    return nc


_CACHE: dict = {}


def get_nc():
    if "nc" not in _CACHE:
        _CACHE["nc"] = build_nc()
    return _CACHE["nc"]


def _pack(a):
    """[512, X] -> [128, 4*X]: k-tile k lands at columns [k*X, (k+1)*X)."""
    d, x = a.shape
    assert d == D
    return np.ascontiguousarray(
        a.reshape(KT4, P, x).transpose(1, 0, 2).reshape(P, KT4 * x)
    )


def make_in_maps(x, y, W_q, W_k, W_v, W_o):
    bf = ml_dtypes.bfloat16
    x = np.ascontiguousarray(x, dtype=np.float32)
    y = np.ascontiguousarray(y, dtype=np.float32)
    wq = _pack((np.asarray(W_q, np.float32) * np.float32(DEPTH**-0.5)).astype(bf))
    wk = _pack(np.asarray(W_k, dtype=np.float32).astype(bf))
    wv = _pack(np.asarray(W_v, dtype=np.float32).astype(bf))
    wo = _pack(np.asarray(W_o, dtype=np.float32).astype(bf))
    in_maps = []
    for b in range(B):
        yT = y[b].T.astype(bf)
        ytA = _pack(yT[:, :HKB])
        ytB = _pack(yT[:, HKB:])
        for half in range(2):
            in_maps.append(
                {
                    "xT": _pack(x[b, half * SQ : (half + 1) * SQ, :].T.astype(bf)),
                    "ytA": ytA,
                    "ytB": ytB,
                    "wq": wq,
                    "wk": wk,
                    "wv": wv,
                    "wo": wo,
                }
            )
    return in_maps


def assemble_out(results):
    out = np.empty((B, S, D), np.float32)
    for c in range(N_CORES):
        b, half = c // 2, c % 2
        out[b, half * SQ : (half + 1) * SQ, :] = results[c]["out"].astype(
            np.float32
        )
    return out


def kernel(x, y, W_q, W_k, W_v, W_o):
    nc = get_nc()
    in_maps = make_in_maps(x, y, W_q, W_k, W_v, W_o)
    res = run_bass_kernel_spmd(nc, in_maps, core_ids=list(range(N_CORES)))
    return assemble_out(res.results)
